# revision 30
# baseline (speedup 1.0000x reference)
"""Multi-head spiking (ReLU) attention on 8 Trainium2 NeuronCores.

Sharding: core c -> (batch b = c//4, head-group g = c%4 of 4 heads).
Host pre-transposes x[b] and slices wq/wk/wv column-wise, wo row-wise.
Each core computes its 4 heads' attention matrix (one of the two graded
outputs) and a rank-256 partial of the output projection; the host sums
the four partials per batch and adds bo.

Device dataflow per core:
  phase 1 (fp32r matmuls): qT/kT [dq,S] projections (evicted to fp16),
    v [S,dq] (fp16).
  phase 2, interleaved per head-pair so attn-store DMA overlaps compute:
    pass1: logits[q,k] (row-paired K=64 fp16 matmuls, N=1024)
           -> ReLU evict -> attn output DMA   (DMA-heavy)
    pass2: logits[k,q] -> ReLU -> fp16 attnT  (PE/evict-heavy)
    ctx^T [dq,S]: v^T @ attnT, 2-head column tiling
  phase 3 (interleaved at the tail): out_part = ctx @ wo (fp16, N=1024).

PSUM is organised as [128,1024] two-bank units so each ACT/DVE eviction
covers 1024 elements (evictions and attn-store DMA are the two
bottleneck resources; TensorE alternates between them).
"""

from dataclasses import dataclass

import numpy as np

import concourse.bass as bass
import concourse.tile as tile
import concourse.mybir as mybir
from concourse import bacc
from concourse.bass_utils import run_bass_kernel_spmd

# Full-problem constants (hardcoded per harness contract).
B, S, D, H = 2, 2048, 1024, 16
DEPTH = D // H  # 64
N_CORES = 8
GROUPS = N_CORES // B       # 4 head-groups
HPC = H // GROUPS           # 4 heads per core
DQ = HPC * DEPTH            # 256 projected dims per core

F32 = mybir.dt.float32
F32R = mybir.dt.float32r
F16 = mybir.dt.float16
RELU = mybir.ActivationFunctionType.Relu


@dataclass(frozen=True)
class Cfg:
    s: int = S       # sequence length
    d: int = D       # model dim
    dq: int = DQ     # per-core projected dims (HPC*64)

    @property
    def ko(self):
        return self.d // 128   # contraction chunks for projections

    @property
    def mq(self):
        return self.dq // 128  # head-pair chunks (2 heads of depth 64 each)

    @property
    def s5(self):
        return self.s // 512

    @property
    def s1(self):
        return self.s // 128


class EvictBalancer:
    """Distribute PSUM->SBUF evictions between ScalarE and VectorE ~5:4
    (ratio of their element rates)."""

    def __init__(self, nc):
        self.nc = nc
        self.i = 0

    def _use_act(self):
        self.i += 1
        return self.i % 9 < 5

    def relu(self, out, psum, eng=None):
        use_act = self._use_act() if eng is None else (eng == "act")
        if use_act:
            self.nc.scalar.activation(out, psum, RELU)
        else:
            self.nc.vector.tensor_scalar_max(out, psum, 0.0)

    def copy(self, out, psum, eng=None):
        use_act = self._use_act() if eng is None else (eng == "act")
        if use_act:
            self.nc.scalar.copy(out, psum)
        else:
            self.nc.vector.tensor_copy(out, psum)


def build_nc(cfg: Cfg = Cfg()):
    s, d, dq = cfg.s, cfg.d, cfg.dq
    KO, MQ, S5, S1 = cfg.ko, cfg.mq, cfg.s5, cfg.s1
    S10 = s // 1024

    nc = bacc.Bacc(None, target_bir_lowering=False)

    # fp32r external inputs: bytes are plain fp32; the PE rounds on
    # consumption, and the dtype satisfies the fp32r-rounding verifier
    # without any casting DMA.
    xt = nc.dram_tensor("xt", [d, s], F32R, kind="ExternalInput")
    wq = nc.dram_tensor("wq", [d, dq], F32R, kind="ExternalInput")
    bq = nc.dram_tensor("bq", [dq], F32, kind="ExternalInput")
    wk = nc.dram_tensor("wk", [d, dq], F32R, kind="ExternalInput")
    bk = nc.dram_tensor("bk", [dq], F32, kind="ExternalInput")
    wv = nc.dram_tensor("wv", [d, dq], F32R, kind="ExternalInput")
    bv = nc.dram_tensor("bv", [dq], F32, kind="ExternalInput")
    wo = nc.dram_tensor("wo", [dq, d], F32, kind="ExternalInput")
    attn_p = nc.dram_tensor("attn_p", [2 * MQ, s, s], F32, kind="ExternalOutput")
    out_p = nc.dram_tensor("out_p", [s, d], F32, kind="ExternalOutput")

    with tile.TileContext(nc) as tc:
        ev = EvictBalancer(nc)
        with tc.tile_pool(name="persist", bufs=1) as pp:
            qT = pp.tile([128, MQ, s], F16)
            kT = pp.tile([128, MQ, s], F16)
            vv = pp.tile([128, S1, dq], F16)
            ctxT = pp.tile([128, MQ, s], F16)
            wo_sb = pp.tile([128, MQ, d], F16)
            bq_sb = pp.tile([128, MQ], F32)
            bk_sb = pp.tile([128, MQ], F32)
            bv_sb = pp.tile([128, dq], F32)


            # ---- Phase 1: projections (fp32r) ----
            with (
                tc.tile_pool(name="xw", bufs=1) as xw,
                tc.tile_pool(name="ps_proj", bufs=3, space="PSUM") as psp,
                tc.tile_pool(name="ps_vproj", bufs=2, space="PSUM") as psv,
            ):
                xt_sb = xw.tile([128, KO, s], F32R)
                wq_sb = xw.tile([128, KO, dq], F32R)
                wk_sb = xw.tile([128, KO, dq], F32R)
                wv_sb = xw.tile([128, KO, dq], F32R)
                nc.sync.dma_start(
                    out=xt_sb, in_=xt.rearrange("(o p) t -> p o t", p=128)
                )
                nc.gpsimd.dma_start(
                    out=wq_sb, in_=wq.rearrange("(o p) m -> p o m", p=128)
                )
                nc.gpsimd.dma_start(
                    out=wk_sb, in_=wk.rearrange("(o p) m -> p o m", p=128)
                )
                nc.gpsimd.dma_start(
                    out=wv_sb, in_=wv.rearrange("(o p) m -> p o m", p=128)
                )
                nc.gpsimd.dma_start(out=bq_sb, in_=bq.rearrange("(c p) -> p c", p=128))
                nc.gpsimd.dma_start(out=bk_sb, in_=bk.rearrange("(c p) -> p c", p=128))
                nc.gpsimd.dma_start(out=bv_sb, in_=bv[None, :].to_broadcast([128, dq]))
                # wo cast fp32->fp16 during the (SWDGE) load
                nc.gpsimd.dma_start(
                    out=wo_sb, in_=wo.rearrange("(c p) e -> p c e", p=128)
                )

                # qT / kT: [dq-part, tokens] fp16, 1024-wide PSUM units
                for w_sb, b_sb, dst in ((wq_sb, bq_sb, qT), (wk_sb, bk_sb, kT)):
                    for m in range(MQ):
                        for t in range(S10):
                            ps = psp.tile([128, 1024], F32, tag="proj")
                            for half in range(2):
                                tt = 2 * t + half
                                for o in range(KO):
                                    nc.tensor.matmul(
                                        ps[:, half * 512:(half + 1) * 512],
                                        lhsT=w_sb[:, o, m * 128:(m + 1) * 128],
                                        rhs=xt_sb[:, o, tt * 512:(tt + 1) * 512],
                                        start=(o == 0),
                                        stop=(o == KO - 1),
                                    )
                            nc.vector.tensor_scalar_add(
                                out=dst[:, m, t * 1024:(t + 1) * 1024],
                                in0=ps,
                                scalar1=b_sb[:, m, None],
                            )

                # v: [token-part, dq] fp16
                for t in range(S1):
                    ps = psv.tile([128, dq], F32, tag="vproj")
                    for o in range(KO):
                        nc.tensor.matmul(
                            ps,
                            lhsT=xt_sb[:, o, t * 128:(t + 1) * 128],
                            rhs=wv_sb[:, o, :],
                            start=(o == 0),
                            stop=(o == KO - 1),
                        )
                    nc.vector.tensor_tensor(
                        vv[:, t, :], ps, bv_sb, mybir.AluOpType.add
                    )

            # ---- Phase 2+3: attention, interleaved with output proj ----
            with (
                tc.tile_pool(name="ps_att", bufs=3, space="PSUM") as psa,
                tc.tile_pool(name="ps_misc", bufs=1, space="PSUM") as psm,
                tc.tile_pool(name="attn_rows", bufs=5) as rowpool,
                tc.tile_pool(name="attnT", bufs=2) as atpool,
                tc.tile_pool(name="out_rows", bufs=3) as outpool,
            ):
                at_tiles = {}
                warm_i = [0]

                def warmup(n=16):
                    # dense, dependency-free PE burst: flips the HAM clock
                    # gate to 8/8 after a phase-boundary stall. Output is
                    # never read.
                    i = warm_i[0]
                    warm_i[0] += 1
                    ps = psm.tile([128, 1024], F32, tag="big",
                                  name=f"warm_{i}")
                    for k in range(n):
                        nc.tensor.matmul(
                            ps[:, :512],
                            lhsT=qT[:, 0, 0:128],
                            rhs=kT[:, 0, 0:512],
                        )

                def pass1_qc(pair, qc):
                    # one row of 128 q-tokens for both heads of the pair;
                    # fp16 N=1024 matmuls, row-paired across heads
                    rows_sb = [
                        rowpool.tile([128, s], F32, tag=f"row{h2}",
                                     name=f"row{pair}_{qc}_{h2}")
                        for h2 in range(2)
                    ]
                    for kc in range(S10):
                        pss = [
                            psa.tile([128, 1024], F32, tag="att",
                                     name=f"p1_{pair}_{qc}_{kc}_{h2}")
                            for h2 in range(2)
                        ]
                        for half in range(2):
                            kk = 2 * kc + half
                            for h2 in range(2):
                                rr = slice(64 * h2, 64 * (h2 + 1))
                                for mh in range(2):
                                    # 64x64 array tiling: tiles (64h2, 64mh)
                                    # all four run concurrently
                                    q0 = qc * 128 + 64 * mh
                                    nc.tensor.matmul(
                                        pss[h2][64 * mh:64 * (mh + 1),
                                                half * 512:(half + 1) * 512],
                                        lhsT=qT[rr, pair, q0:q0 + 64],
                                        rhs=kT[rr, pair, kk * 512:(kk + 1) * 512],
                                    )
                        for h2 in range(2):
                            ev.relu(
                                rows_sb[h2][:, kc * 1024:(kc + 1) * 1024],
                                pss[h2],
                            )
                    for h2 in range(2):
                        nc.sync.dma_start(
                            out=attn_p[2 * pair + h2, qc * 128:(qc + 1) * 128, :],
                            in_=rows_sb[h2],
                        )

                def pass2_j(pair, j):
                    # attnT for q-slice j*512, all 16 k-chunks, both heads
                    at = atpool.tile([128, 2, S1, 512], F16, tag="at",
                                     name=f"at_{pair}_{j}")
                    at_tiles[(pair, j)] = at
                    for kc in range(S1 // 2):
                        pss = [
                            psa.tile([128, 1024], F32, tag="att",
                                     name=f"p2_{pair}_{j}_{kc}_{h2}")
                            for h2 in range(2)
                        ]
                        for half in range(2):
                            kk = 2 * kc + half
                            for h2 in range(2):
                                rr = slice(64 * h2, 64 * (h2 + 1))
                                for mh in range(2):
                                    k0 = kk * 128 + 64 * mh
                                    nc.tensor.matmul(
                                        pss[h2][64 * mh:64 * (mh + 1),
                                                half * 512:(half + 1) * 512],
                                        lhsT=kT[rr, pair, k0:k0 + 64],
                                        rhs=qT[rr, pair, j * 512:(j + 1) * 512],
                                    )
                        for h2 in range(2):
                            ev.relu(
                                at[:, h2, 2 * kc:2 * kc + 2, :].rearrange(
                                    "p a b -> p (a b)"
                                ),
                                pss[h2],
                            )

                def ctx_j2(pair, j2):
                    # one [128,1024] unit = two q-chunks of 512
                    ps = psm.tile([128, 1024], F32, tag="big",
                                  name=f"ctx_{pair}_{j2}")
                    for jj in range(2):
                        at = at_tiles.pop((pair, 2 * j2 + jj))
                        sl = slice(jj * 512, (jj + 1) * 512)
                        for kc in range(S1):
                            for h2 in range(2):
                                nc.tensor.matmul(
                                    ps[64 * h2:64 * (h2 + 1), sl],
                                    lhsT=vv[:, kc, pair * 128 + 64 * h2:
                                            pair * 128 + 64 * (h2 + 1)],
                                    rhs=at[:, h2, kc, :],
                                    start=(kc == 0),
                                    stop=(kc == S1 - 1),
                                )
                    ev.copy(ctxT[:, pair, j2 * 1024:(j2 + 1) * 1024], ps)

                def outproj_qc(qc):
                    orow = outpool.tile([128, d], F32, tag="orow",
                                        name=f"orow_{qc}")
                    EW = min(1024, d)
                    for eu in range(d // EW):
                        ps = psm.tile([128, EW], F32, tag="big",
                                      name=f"po_{qc}_{eu}")
                        for e2 in range(EW // 512):
                            e = (EW // 512) * eu + e2
                            for m in range(MQ):
                                nc.tensor.matmul(
                                    ps[:, e2 * 512:(e2 + 1) * 512],
                                    lhsT=ctxT[:, m, qc * 128:(qc + 1) * 128],
                                    rhs=wo_sb[:, m, e * 512:(e + 1) * 512],
                                    start=(m == 0),
                                    stop=(m == MQ - 1),
                                )
                        ev.copy(orow[:, eu * EW:(eu + 1) * EW], ps)
                    nc.sync.dma_start(
                        out=out_p[qc * 128:(qc + 1) * 128, :], in_=orow
                    )

                # Interleave: pass1 paces the DMA; pass2/ctx/outproj slot
                # between its q-chunks to keep PE/ACT/DVE busy.
                for pair in range(MQ):
                    warmup()
                    blocks = []
                    for j in range(S5):
                        blocks.append(("p2", j))
                        if j % 2 == 1:
                            blocks.append(("ctx", j // 2))
                            if pair == MQ - 1:
                                # out-proj q-chunks whose ctxT columns are
                                # complete once both pairs' ctx j-half done
                                half = j // 2
                                nqc = S1 // (S5 // 2)
                                for oq in range(half * nqc, (half + 1) * nqc):
                                    blocks.append(("out", oq))
                    emitted = 0
                    for qc in range(S1):
                        pass1_qc(pair, qc)
                        want = (qc + 1) * len(blocks) // S1
                        while emitted < want:
                            kind, idx = blocks[emitted]
                            emitted += 1
                            if kind == "p2":
                                pass2_j(pair, idx)
                            elif kind == "ctx":
                                ctx_j2(pair, idx)
                            else:
                                outproj_qc(idx)

    nc.finalize()
    return nc


_NC_CACHE = {}


def get_nc(cfg: Cfg = Cfg()):
    if cfg not in _NC_CACHE:
        _NC_CACHE[cfg] = build_nc(cfg)
    return _NC_CACHE[cfg]


LAST_RESULT = None  # BassKernelResults of the most recent kernel() call


def make_in_maps(x, wq, bq, wk, bk, wv, bv, wo):
    scale = 1.0 / np.sqrt(np.float32(DEPTH))
    in_maps = []
    for core in range(N_CORES):
        b, g = divmod(core, GROUPS)
        sl = slice(g * DQ, (g + 1) * DQ)
        in_maps.append({
            "xt": np.ascontiguousarray(x[b].T),
            "wq": np.ascontiguousarray(wq[:, sl]) * scale,
            "bq": np.ascontiguousarray(bq[sl]) * scale,
            "wk": np.ascontiguousarray(wk[:, sl]),
            "bk": np.ascontiguousarray(bk[sl]),
            "wv": np.ascontiguousarray(wv[:, sl]),
            "bv": np.ascontiguousarray(bv[sl]),
            "wo": np.ascontiguousarray(wo[sl, :]),
        })
    return in_maps


def kernel(x, wq, bq, wk, bk, wv, bv, wo, bo, _trace=False):
    global LAST_RESULT
    arrs = [np.asarray(a, np.float32) for a in (x, wq, bq, wk, bk, wv, bv, wo)]
    bo = np.asarray(bo, np.float32)
    in_maps = make_in_maps(*arrs)

    nc = get_nc()
    res = run_bass_kernel_spmd(
        nc, in_maps, core_ids=list(range(N_CORES)), trace=_trace
    )
    LAST_RESULT = res

    attn = np.empty((B, H, S, S), np.float32)
    out = np.broadcast_to(bo, (B, S, D)).copy()
    for core in range(N_CORES):
        b, g = divmod(core, GROUPS)
        r = res.results[core]
        attn[b, g * HPC:(g + 1) * HPC] = r["attn_p"]
        out[b] += r["out_p"]
    return out, attn


# revision 32
# speedup vs baseline: 1.0812x; 1.0812x over previous
"""Multi-head spiking (ReLU) attention on 8 Trainium2 NeuronCores.

Sharding: core c -> (batch b = c//4, head-group g = c%4 of 4 heads).
Host pre-transposes x[b] and slices wq/wk/wv column-wise, wo row-wise.
Each core computes its 4 heads' attention matrix (one of the two graded
outputs) and a rank-256 partial of the output projection; the host sums
the four partials per batch and adds bo.

Device dataflow per core:
  phase 1 (fp32r matmuls): qT/kT [dq,S] projections (evicted to fp16),
    v [S,dq] (fp16).
  phase 2, interleaved per head-pair so attn-store DMA overlaps compute:
    pass1: logits[q,k] (row-paired K=64 fp16 matmuls, N=1024)
           -> ReLU evict -> attn output DMA   (DMA-heavy)
    pass2: logits[k,q] -> ReLU -> fp16 attnT  (PE/evict-heavy)
    ctx^T [dq,S]: v^T @ attnT, 2-head column tiling
  phase 3 (interleaved at the tail): out_part = ctx @ wo (fp16, N=1024).

PSUM is organised as [128,1024] two-bank units so each ACT/DVE eviction
covers 1024 elements (evictions and attn-store DMA are the two
bottleneck resources; TensorE alternates between them).
"""

from dataclasses import dataclass

import numpy as np

import concourse.bass as bass
import concourse.tile as tile
import concourse.mybir as mybir
from concourse import bacc
from concourse.bass_utils import run_bass_kernel_spmd

# Full-problem constants (hardcoded per harness contract).
B, S, D, H = 2, 2048, 1024, 16
DEPTH = D // H  # 64
N_CORES = 8
GROUPS = N_CORES // B       # 4 head-groups
HPC = H // GROUPS           # 4 heads per core
DQ = HPC * DEPTH            # 256 projected dims per core

F32 = mybir.dt.float32
F32R = mybir.dt.float32r
F16 = mybir.dt.float16
RELU = mybir.ActivationFunctionType.Relu


@dataclass(frozen=True)
class Cfg:
    s: int = S       # sequence length
    d: int = D       # model dim
    dq: int = DQ     # per-core projected dims (HPC*64)

    @property
    def ko(self):
        return self.d // 128   # contraction chunks for projections

    @property
    def mq(self):
        return self.dq // 128  # head-pair chunks (2 heads of depth 64 each)

    @property
    def s5(self):
        return self.s // 512

    @property
    def s1(self):
        return self.s // 128


class EvictBalancer:
    """Distribute PSUM->SBUF evictions between ScalarE and VectorE ~5:4
    (ratio of their element rates)."""

    def __init__(self, nc):
        self.nc = nc
        self.i = 0

    def _use_act(self):
        self.i += 1
        return self.i % 9 < 5

    def relu(self, out, psum, eng=None):
        use_act = self._use_act() if eng is None else (eng == "act")
        if use_act:
            self.nc.scalar.activation(out, psum, RELU)
        else:
            self.nc.vector.tensor_scalar_max(out, psum, 0.0)

    def copy(self, out, psum, eng=None):
        use_act = self._use_act() if eng is None else (eng == "act")
        if use_act:
            self.nc.scalar.copy(out, psum)
        else:
            self.nc.vector.tensor_copy(out, psum)


def build_nc(cfg: Cfg = Cfg()):
    s, d, dq = cfg.s, cfg.d, cfg.dq
    KO, MQ, S5, S1 = cfg.ko, cfg.mq, cfg.s5, cfg.s1
    S10 = s // 1024

    nc = bacc.Bacc(None, target_bir_lowering=False)

    # fp32r external inputs: bytes are plain fp32; the PE rounds on
    # consumption, and the dtype satisfies the fp32r-rounding verifier
    # without any casting DMA.
    xt = nc.dram_tensor("xt", [d, s], F32R, kind="ExternalInput")
    wq = nc.dram_tensor("wq", [d, dq], F32R, kind="ExternalInput")
    bq = nc.dram_tensor("bq", [dq], F32, kind="ExternalInput")
    wk = nc.dram_tensor("wk", [d, dq], F32R, kind="ExternalInput")
    bk = nc.dram_tensor("bk", [dq], F32, kind="ExternalInput")
    wv = nc.dram_tensor("wv", [d, dq], F32R, kind="ExternalInput")
    bv = nc.dram_tensor("bv", [dq], F32, kind="ExternalInput")
    wo = nc.dram_tensor("wo", [dq, d], F32, kind="ExternalInput")
    attn_p = nc.dram_tensor("attn_p", [2 * MQ, s, s], F32, kind="ExternalOutput")
    out_p = nc.dram_tensor("out_p", [s, d], F32, kind="ExternalOutput")

    with tile.TileContext(nc) as tc:
        ev = EvictBalancer(nc)
        with tc.tile_pool(name="persist", bufs=1) as pp:
            qT = pp.tile([128, MQ, s], F16)
            kT = pp.tile([128, MQ, s], F16)
            vv = pp.tile([128, S1, dq], F16)
            ctxT = pp.tile([128, MQ, s], F16)
            wo_sb = pp.tile([128, MQ, d], F16)
            bq_sb = pp.tile([128, MQ], F32)
            bk_sb = pp.tile([128, MQ], F32)
            bv_sb = pp.tile([128, dq], F32)


            # ---- Phase 1: projections (fp32r) ----
            with (
                tc.tile_pool(name="xw", bufs=1) as xw,
                tc.tile_pool(name="ps_proj", bufs=3, space="PSUM") as psp,
                tc.tile_pool(name="ps_vproj", bufs=2, space="PSUM") as psv,
            ):
                xt_sb = xw.tile([128, KO, s], F32R)
                wq_sb = xw.tile([128, KO, dq], F32R)
                wk_sb = xw.tile([128, KO, dq], F32R)
                wv_sb = xw.tile([128, KO, dq], F32R)
                nc.sync.dma_start(
                    out=xt_sb, in_=xt.rearrange("(o p) t -> p o t", p=128)
                )
                nc.gpsimd.dma_start(
                    out=wq_sb, in_=wq.rearrange("(o p) m -> p o m", p=128)
                )
                nc.gpsimd.dma_start(
                    out=wk_sb, in_=wk.rearrange("(o p) m -> p o m", p=128)
                )
                nc.gpsimd.dma_start(
                    out=wv_sb, in_=wv.rearrange("(o p) m -> p o m", p=128)
                )
                nc.gpsimd.dma_start(out=bq_sb, in_=bq.rearrange("(c p) -> p c", p=128))
                nc.gpsimd.dma_start(out=bk_sb, in_=bk.rearrange("(c p) -> p c", p=128))
                nc.gpsimd.dma_start(out=bv_sb, in_=bv[None, :].to_broadcast([128, dq]))
                # wo cast fp32->fp16 during the (SWDGE) load
                nc.gpsimd.dma_start(
                    out=wo_sb, in_=wo.rearrange("(c p) e -> p c e", p=128)
                )

                # qT / kT: [dq-part, tokens] fp16, 1024-wide PSUM units
                for w_sb, b_sb, dst in ((wq_sb, bq_sb, qT), (wk_sb, bk_sb, kT)):
                    for m in range(MQ):
                        for t in range(S10):
                            ps = psp.tile([128, 1024], F32, tag="proj")
                            for half in range(2):
                                tt = 2 * t + half
                                for o in range(KO):
                                    nc.tensor.matmul(
                                        ps[:, half * 512:(half + 1) * 512],
                                        lhsT=w_sb[:, o, m * 128:(m + 1) * 128],
                                        rhs=xt_sb[:, o, tt * 512:(tt + 1) * 512],
                                        start=(o == 0),
                                        stop=(o == KO - 1),
                                    )
                            nc.vector.tensor_scalar_add(
                                out=dst[:, m, t * 1024:(t + 1) * 1024],
                                in0=ps,
                                scalar1=b_sb[:, m, None],
                            )

                # v: [token-part, dq] fp16
                for t in range(S1):
                    ps = psv.tile([128, dq], F32, tag="vproj")
                    for o in range(KO):
                        nc.tensor.matmul(
                            ps,
                            lhsT=xt_sb[:, o, t * 128:(t + 1) * 128],
                            rhs=wv_sb[:, o, :],
                            start=(o == 0),
                            stop=(o == KO - 1),
                        )
                    nc.vector.tensor_tensor(
                        vv[:, t, :], ps, bv_sb, mybir.AluOpType.add
                    )

            # ---- Phase 2+3: attention, interleaved with output proj ----
            with (
                tc.tile_pool(name="ps_att", bufs=3, space="PSUM") as psa,
                tc.tile_pool(name="ps_misc", bufs=1, space="PSUM") as psm,
                tc.tile_pool(name="attn_rows", bufs=5) as rowpool,
                tc.tile_pool(name="attnT", bufs=2) as atpool,
                tc.tile_pool(name="out_rows", bufs=3) as outpool,
            ):
                at_tiles = {}
                warm_i = [0]

                def warmup(n=16):
                    # dense, dependency-free PE burst: flips the HAM clock
                    # gate to 8/8 after a phase-boundary stall. Output is
                    # never read.
                    i = warm_i[0]
                    warm_i[0] += 1
                    ps = psm.tile([128, 1024], F32, tag="big",
                                  name=f"warm_{i}")
                    for k in range(n):
                        nc.tensor.matmul(
                            ps[:, :512],
                            lhsT=qT[:, 0, 0:128],
                            rhs=kT[:, 0, 0:512],
                        )

                def pass1_qc(pair, qc):
                    # one row of 128 q-tokens for both heads of the pair;
                    # fp16 N=1024 matmuls, row-paired across heads
                    rows_sb = [
                        rowpool.tile([128, s], F32, tag=f"row{h2}",
                                     name=f"row{pair}_{qc}_{h2}")
                        for h2 in range(2)
                    ]
                    for kc in range(S10):
                        pss = [
                            psa.tile([128, 1024], F32, tag="att",
                                     name=f"p1_{pair}_{qc}_{kc}_{h2}")
                            for h2 in range(2)
                        ]
                        for half in range(2):
                            kk = 2 * kc + half
                            for h2 in range(2):
                                rr = slice(64 * h2, 64 * (h2 + 1))
                                for mh in range(2):
                                    # 64x64 array tiling: tiles (64h2, 64mh)
                                    # all four run concurrently
                                    q0 = qc * 128 + 64 * mh
                                    nc.tensor.matmul(
                                        pss[h2][64 * mh:64 * (mh + 1),
                                                half * 512:(half + 1) * 512],
                                        lhsT=qT[rr, pair, q0:q0 + 64],
                                        rhs=kT[rr, pair, kk * 512:(kk + 1) * 512],
                                    )
                        for h2 in range(2):
                            ev.relu(
                                rows_sb[h2][:, kc * 1024:(kc + 1) * 1024],
                                pss[h2],
                            )
                    for h2 in range(2):
                        nc.sync.dma_start(
                            out=attn_p[2 * pair + h2, qc * 128:(qc + 1) * 128, :],
                            in_=rows_sb[h2],
                        )

                def pass2_j(pair, j):
                    # attnT for q-slice j*512, all 16 k-chunks, both heads
                    at = atpool.tile([128, 2, S1, 512], F16, tag="at",
                                     name=f"at_{pair}_{j}")
                    at_tiles[(pair, j)] = at
                    for kc in range(S1 // 2):
                        pss = [
                            psa.tile([128, 1024], F32, tag="att",
                                     name=f"p2_{pair}_{j}_{kc}_{h2}")
                            for h2 in range(2)
                        ]
                        for half in range(2):
                            kk = 2 * kc + half
                            for h2 in range(2):
                                rr = slice(64 * h2, 64 * (h2 + 1))
                                for mh in range(2):
                                    k0 = kk * 128 + 64 * mh
                                    nc.tensor.matmul(
                                        pss[h2][64 * mh:64 * (mh + 1),
                                                half * 512:(half + 1) * 512],
                                        lhsT=kT[rr, pair, k0:k0 + 64],
                                        rhs=qT[rr, pair, j * 512:(j + 1) * 512],
                                    )
                        for h2 in range(2):
                            ev.relu(
                                at[:, h2, 2 * kc:2 * kc + 2, :].rearrange(
                                    "p a b -> p (a b)"
                                ),
                                pss[h2],
                            )

                def ctx_j2(pair, j2):
                    # one [128,1024] unit = two q-chunks of 512
                    ps = psm.tile([128, 1024], F32, tag="big",
                                  name=f"ctx_{pair}_{j2}")
                    for jj in range(2):
                        at = at_tiles.pop((pair, 2 * j2 + jj))
                        sl = slice(jj * 512, (jj + 1) * 512)
                        for kc in range(S1):
                            for h2 in range(2):
                                nc.tensor.matmul(
                                    ps[64 * h2:64 * (h2 + 1), sl],
                                    lhsT=vv[:, kc, pair * 128 + 64 * h2:
                                            pair * 128 + 64 * (h2 + 1)],
                                    rhs=at[:, h2, kc, :],
                                    start=(kc == 0),
                                    stop=(kc == S1 - 1),
                                )
                    ev.copy(ctxT[:, pair, j2 * 1024:(j2 + 1) * 1024], ps)

                def outproj_qc(qc):
                    orow = outpool.tile([128, d], F32, tag="orow",
                                        name=f"orow_{qc}")
                    EW = min(1024, d)
                    for eu in range(d // EW):
                        ps = psm.tile([128, EW], F32, tag="big",
                                      name=f"po_{qc}_{eu}")
                        for e2 in range(EW // 512):
                            e = (EW // 512) * eu + e2
                            for m in range(MQ):
                                nc.tensor.matmul(
                                    ps[:, e2 * 512:(e2 + 1) * 512],
                                    lhsT=ctxT[:, m, qc * 128:(qc + 1) * 128],
                                    rhs=wo_sb[:, m, e * 512:(e + 1) * 512],
                                    start=(m == 0),
                                    stop=(m == MQ - 1),
                                )
                        ev.copy(orow[:, eu * EW:(eu + 1) * EW], ps)
                    nc.sync.dma_start(
                        out=out_p[qc * 128:(qc + 1) * 128, :], in_=orow
                    )

                # Interleave: pass1 paces the DMA; pass2/ctx/outproj slot
                # between its q-chunks to keep PE/ACT/DVE busy.
                for pair in range(MQ):
                    blocks = []
                    for j in range(S5):
                        blocks.append(("p2", j))
                        if j % 2 == 1:
                            blocks.append(("ctx", j // 2))
                    emitted = 0
                    for qc in range(S1):
                        pass1_qc(pair, qc)
                        want = (qc + 1) * len(blocks) // S1
                        while emitted < want:
                            kind, idx = blocks[emitted]
                            emitted += 1
                            if kind == "p2":
                                pass2_j(pair, idx)
                            else:
                                ctx_j2(pair, idx)
                    if pair == MQ - 1:
                        for qc in range(S1):
                            outproj_qc(qc)

    nc.finalize()
    return nc


_NC_CACHE = {}


def get_nc(cfg: Cfg = Cfg()):
    if cfg not in _NC_CACHE:
        _NC_CACHE[cfg] = build_nc(cfg)
    return _NC_CACHE[cfg]


LAST_RESULT = None  # BassKernelResults of the most recent kernel() call


def make_in_maps(x, wq, bq, wk, bk, wv, bv, wo):
    scale = 1.0 / np.sqrt(np.float32(DEPTH))
    in_maps = []
    for core in range(N_CORES):
        b, g = divmod(core, GROUPS)
        sl = slice(g * DQ, (g + 1) * DQ)
        in_maps.append({
            "xt": np.ascontiguousarray(x[b].T),
            "wq": np.ascontiguousarray(wq[:, sl]) * scale,
            "bq": np.ascontiguousarray(bq[sl]) * scale,
            "wk": np.ascontiguousarray(wk[:, sl]),
            "bk": np.ascontiguousarray(bk[sl]),
            "wv": np.ascontiguousarray(wv[:, sl]),
            "bv": np.ascontiguousarray(bv[sl]),
            "wo": np.ascontiguousarray(wo[sl, :]),
        })
    return in_maps


def kernel(x, wq, bq, wk, bk, wv, bv, wo, bo, _trace=False):
    global LAST_RESULT
    arrs = [np.asarray(a, np.float32) for a in (x, wq, bq, wk, bk, wv, bv, wo)]
    bo = np.asarray(bo, np.float32)
    in_maps = make_in_maps(*arrs)

    nc = get_nc()
    res = run_bass_kernel_spmd(
        nc, in_maps, core_ids=list(range(N_CORES)), trace=_trace
    )
    LAST_RESULT = res

    attn = np.empty((B, H, S, S), np.float32)
    out = np.broadcast_to(bo, (B, S, D)).copy()
    for core in range(N_CORES):
        b, g = divmod(core, GROUPS)
        r = res.results[core]
        attn[b, g * HPC:(g + 1) * HPC] = r["attn_p"]
        out[b] += r["out_p"]
    return out, attn


# revision 33
# speedup vs baseline: 1.2898x; 1.1929x over previous
"""Multi-head spiking (ReLU) attention on 8 Trainium2 NeuronCores.

Sharding: core c -> (batch b = c//4, head-group g = c%4 of 4 heads).
Host pre-transposes x[b] and slices wq/wk/wv column-wise, wo row-wise.
Each core computes its 4 heads' attention matrix (one of the two graded
outputs) and a rank-256 partial of the output projection; the host sums
the four partials per batch and adds bo.

Device dataflow per core:
  phase 1 (fp32r matmuls): qT/kT [dq,S] projections (evicted to fp16),
    v [S,dq] (fp16).
  phase 2, interleaved per head-pair so attn-store DMA overlaps compute:
    pass1: logits[q,k] (row-paired K=64 fp16 matmuls, N=1024)
           -> ReLU evict -> attn output DMA   (DMA-heavy)
    pass2: logits[k,q] -> ReLU -> fp16 attnT  (PE/evict-heavy)
    ctx^T [dq,S]: v^T @ attnT, 2-head column tiling
  phase 3 (interleaved at the tail): out_part = ctx @ wo (fp16, N=1024).

PSUM is organised as [128,1024] two-bank units so each ACT/DVE eviction
covers 1024 elements (evictions and attn-store DMA are the two
bottleneck resources; TensorE alternates between them).
"""

from dataclasses import dataclass

import numpy as np

import concourse.bass as bass
import concourse.tile as tile
import concourse.mybir as mybir
from concourse import bacc
from concourse.bass_utils import run_bass_kernel_spmd

# Full-problem constants (hardcoded per harness contract).
B, S, D, H = 2, 2048, 1024, 16
DEPTH = D // H  # 64
N_CORES = 8
GROUPS = N_CORES // B       # 4 head-groups
HPC = H // GROUPS           # 4 heads per core
DQ = HPC * DEPTH            # 256 projected dims per core

F32 = mybir.dt.float32
F32R = mybir.dt.float32r
F16 = mybir.dt.float16
RELU = mybir.ActivationFunctionType.Relu


@dataclass(frozen=True)
class Cfg:
    s: int = S       # sequence length
    d: int = D       # model dim
    dq: int = DQ     # per-core projected dims (HPC*64)

    @property
    def ko(self):
        return self.d // 128   # contraction chunks for projections

    @property
    def mq(self):
        return self.dq // 128  # head-pair chunks (2 heads of depth 64 each)

    @property
    def s5(self):
        return self.s // 512

    @property
    def s1(self):
        return self.s // 128


class EvictBalancer:
    """Distribute PSUM->SBUF evictions between ScalarE and VectorE ~5:4
    (ratio of their element rates)."""

    def __init__(self, nc):
        self.nc = nc
        self.i = 0

    def _use_act(self):
        self.i += 1
        return self.i % 9 < 5

    def relu(self, out, psum, eng=None):
        use_act = self._use_act() if eng is None else (eng == "act")
        if use_act:
            self.nc.scalar.activation(out, psum, RELU)
        else:
            self.nc.vector.tensor_scalar_max(out, psum, 0.0)

    def copy(self, out, psum, eng=None):
        use_act = self._use_act() if eng is None else (eng == "act")
        if use_act:
            self.nc.scalar.copy(out, psum)
        else:
            self.nc.vector.tensor_copy(out, psum)


def build_nc(cfg: Cfg = Cfg()):
    s, d, dq = cfg.s, cfg.d, cfg.dq
    KO, MQ, S5, S1 = cfg.ko, cfg.mq, cfg.s5, cfg.s1
    S10 = s // 1024

    nc = bacc.Bacc(None, target_bir_lowering=False)

    # fp32r external inputs: bytes are plain fp32; the PE rounds on
    # consumption, and the dtype satisfies the fp32r-rounding verifier
    # without any casting DMA.
    xt = nc.dram_tensor("xt", [d, s], F32R, kind="ExternalInput")
    wq = nc.dram_tensor("wq", [d, dq], F32R, kind="ExternalInput")
    bq = nc.dram_tensor("bq", [dq], F32, kind="ExternalInput")
    wk = nc.dram_tensor("wk", [d, dq], F32R, kind="ExternalInput")
    bk = nc.dram_tensor("bk", [dq], F32, kind="ExternalInput")
    wv = nc.dram_tensor("wv", [d, dq], F32R, kind="ExternalInput")
    bv = nc.dram_tensor("bv", [dq], F32, kind="ExternalInput")
    wo = nc.dram_tensor("wo", [dq, d], F32, kind="ExternalInput")
    attn_p = nc.dram_tensor("attn_p", [2 * MQ, s, s], F32, kind="ExternalOutput")
    out_p = nc.dram_tensor("out_p", [s, d], F32, kind="ExternalOutput")

    with tile.TileContext(nc) as tc:
        ev = EvictBalancer(nc)
        with tc.tile_pool(name="persist", bufs=1) as pp:
            qT = pp.tile([128, MQ, s], F16)
            kT = pp.tile([128, MQ, s], F16)
            vv = pp.tile([128, S1, dq], F16)
            ctxT = pp.tile([128, MQ, s], F16)
            wo_sb = pp.tile([128, MQ, d], F16)
            bq_sb = pp.tile([128, MQ], F32)
            bk_sb = pp.tile([128, MQ], F32)
            bv_sb = pp.tile([128, dq], F32)


            # ---- Phase 1: projections (fp32r) ----
            with (
                tc.tile_pool(name="xw", bufs=1) as xw,
                tc.tile_pool(name="ps_proj", bufs=3, space="PSUM") as psp,
                tc.tile_pool(name="ps_vproj", bufs=2, space="PSUM") as psv,
            ):
                xt_sb = xw.tile([128, KO, s], F32R)
                wq_sb = xw.tile([128, KO, dq], F32R)
                wk_sb = xw.tile([128, KO, dq], F32R)
                wv_sb = xw.tile([128, KO, dq], F32R)
                nc.sync.dma_start(
                    out=xt_sb, in_=xt.rearrange("(o p) t -> p o t", p=128)
                )
                nc.gpsimd.dma_start(
                    out=wq_sb, in_=wq.rearrange("(o p) m -> p o m", p=128)
                )
                nc.gpsimd.dma_start(
                    out=wk_sb, in_=wk.rearrange("(o p) m -> p o m", p=128)
                )
                nc.gpsimd.dma_start(
                    out=wv_sb, in_=wv.rearrange("(o p) m -> p o m", p=128)
                )
                nc.gpsimd.dma_start(out=bq_sb, in_=bq.rearrange("(c p) -> p c", p=128))
                nc.gpsimd.dma_start(out=bk_sb, in_=bk.rearrange("(c p) -> p c", p=128))
                nc.gpsimd.dma_start(out=bv_sb, in_=bv[None, :].to_broadcast([128, dq]))
                # wo cast fp32->fp16 during the (SWDGE) load
                nc.gpsimd.dma_start(
                    out=wo_sb, in_=wo.rearrange("(c p) e -> p c e", p=128)
                )

                # qT / kT: [dq-part, tokens] fp16, 1024-wide PSUM units
                for w_sb, b_sb, dst in ((wq_sb, bq_sb, qT), (wk_sb, bk_sb, kT)):
                    for m in range(MQ):
                        for t in range(S10):
                            ps = psp.tile([128, 1024], F32, tag="proj")
                            for half in range(2):
                                tt = 2 * t + half
                                for o in range(KO):
                                    nc.tensor.matmul(
                                        ps[:, half * 512:(half + 1) * 512],
                                        lhsT=w_sb[:, o, m * 128:(m + 1) * 128],
                                        rhs=xt_sb[:, o, tt * 512:(tt + 1) * 512],
                                        start=(o == 0),
                                        stop=(o == KO - 1),
                                    )
                            nc.vector.tensor_scalar_add(
                                out=dst[:, m, t * 1024:(t + 1) * 1024],
                                in0=ps,
                                scalar1=b_sb[:, m, None],
                            )

                # v: [token-part, dq] fp16
                for t in range(S1):
                    ps = psv.tile([128, dq], F32, tag="vproj")
                    for o in range(KO):
                        nc.tensor.matmul(
                            ps,
                            lhsT=xt_sb[:, o, t * 128:(t + 1) * 128],
                            rhs=wv_sb[:, o, :],
                            start=(o == 0),
                            stop=(o == KO - 1),
                        )
                    nc.vector.tensor_tensor(
                        vv[:, t, :], ps, bv_sb, mybir.AluOpType.add
                    )

            # ---- Phase 2+3: attention, interleaved with output proj ----
            with (
                tc.tile_pool(name="ps_att", bufs=4, space="PSUM") as psa,
                tc.tile_pool(name="attn_rows", bufs=5) as rowpool,
                tc.tile_pool(name="attnT", bufs=2) as atpool,
                tc.tile_pool(name="out_rows", bufs=3) as outpool,
            ):
                at_tiles = {}

                def pass1_qc(pair, qc):
                    # one row of 128 q-tokens for both heads of the pair;
                    # fp16 N=1024 matmuls, row-paired across heads
                    rows_sb = [
                        rowpool.tile([128, s], F32, tag=f"row{h2}",
                                     name=f"row{pair}_{qc}_{h2}")
                        for h2 in range(2)
                    ]
                    for kc in range(S10):
                        pss = [
                            psa.tile([128, 1024], F32, tag="att",
                                     name=f"p1_{pair}_{qc}_{kc}_{h2}")
                            for h2 in range(2)
                        ]
                        for half in range(2):
                            kk = 2 * kc + half
                            for h2 in range(2):
                                rr = slice(64 * h2, 64 * (h2 + 1))
                                for mh in range(2):
                                    # 64x64 array tiling: tiles (64h2, 64mh)
                                    # all four run concurrently
                                    q0 = qc * 128 + 64 * mh
                                    nc.tensor.matmul(
                                        pss[h2][64 * mh:64 * (mh + 1),
                                                half * 512:(half + 1) * 512],
                                        lhsT=qT[rr, pair, q0:q0 + 64],
                                        rhs=kT[rr, pair, kk * 512:(kk + 1) * 512],
                                    )
                        for h2 in range(2):
                            ev.relu(
                                rows_sb[h2][:, kc * 1024:(kc + 1) * 1024],
                                pss[h2],
                            )
                    for h2 in range(2):
                        nc.sync.dma_start(
                            out=attn_p[2 * pair + h2, qc * 128:(qc + 1) * 128, :],
                            in_=rows_sb[h2],
                        )

                def pass2_j(pair, j):
                    # attnT for q-slice j*512, all 16 k-chunks, both heads
                    at = atpool.tile([128, 2, S1, 512], F16, tag="at",
                                     name=f"at_{pair}_{j}")
                    at_tiles[(pair, j)] = at
                    for kc in range(S1 // 2):
                        pss = [
                            psa.tile([128, 1024], F32, tag="att",
                                     name=f"p2_{pair}_{j}_{kc}_{h2}")
                            for h2 in range(2)
                        ]
                        for half in range(2):
                            kk = 2 * kc + half
                            for h2 in range(2):
                                rr = slice(64 * h2, 64 * (h2 + 1))
                                for mh in range(2):
                                    k0 = kk * 128 + 64 * mh
                                    nc.tensor.matmul(
                                        pss[h2][64 * mh:64 * (mh + 1),
                                                half * 512:(half + 1) * 512],
                                        lhsT=kT[rr, pair, k0:k0 + 64],
                                        rhs=qT[rr, pair, j * 512:(j + 1) * 512],
                                    )
                        for h2 in range(2):
                            ev.relu(
                                at[:, h2, 2 * kc:2 * kc + 2, :].rearrange(
                                    "p a b -> p (a b)"
                                ),
                                pss[h2],
                            )

                def ctx_j2(pair, j2):
                    # one [128,1024] unit = two q-chunks of 512
                    ps = psa.tile([128, 1024], F32, tag="att",
                                  name=f"ctx_{pair}_{j2}")
                    for jj in range(2):
                        at = at_tiles.pop((pair, 2 * j2 + jj))
                        sl = slice(jj * 512, (jj + 1) * 512)
                        for kc in range(S1):
                            for h2 in range(2):
                                nc.tensor.matmul(
                                    ps[64 * h2:64 * (h2 + 1), sl],
                                    lhsT=vv[:, kc, pair * 128 + 64 * h2:
                                            pair * 128 + 64 * (h2 + 1)],
                                    rhs=at[:, h2, kc, :],
                                    start=(kc == 0),
                                    stop=(kc == S1 - 1),
                                )
                    ev.copy(ctxT[:, pair, j2 * 1024:(j2 + 1) * 1024], ps)

                def outproj_qc(qc):
                    orow = outpool.tile([128, d], F32, tag="orow",
                                        name=f"orow_{qc}")
                    EW = min(1024, d)
                    for eu in range(d // EW):
                        ps = psa.tile([128, EW], F32, tag="att",
                                      name=f"po_{qc}_{eu}")
                        for e2 in range(EW // 512):
                            e = (EW // 512) * eu + e2
                            for m in range(MQ):
                                nc.tensor.matmul(
                                    ps[:, e2 * 512:(e2 + 1) * 512],
                                    lhsT=ctxT[:, m, qc * 128:(qc + 1) * 128],
                                    rhs=wo_sb[:, m, e * 512:(e + 1) * 512],
                                    start=(m == 0),
                                    stop=(m == MQ - 1),
                                )
                        ev.copy(orow[:, eu * EW:(eu + 1) * EW], ps)
                    nc.sync.dma_start(
                        out=out_p[qc * 128:(qc + 1) * 128, :], in_=orow
                    )

                # Interleave: pass1 paces the DMA; pass2/ctx/outproj slot
                # between its q-chunks to keep PE/ACT/DVE busy.
                for pair in range(MQ):
                    blocks = []
                    for j in range(S5):
                        blocks.append(("p2", j))
                        if j % 2 == 1:
                            blocks.append(("ctx", j // 2))
                    emitted = 0
                    for qc in range(S1):
                        pass1_qc(pair, qc)
                        want = (qc + 1) * len(blocks) // S1
                        while emitted < want:
                            kind, idx = blocks[emitted]
                            emitted += 1
                            if kind == "p2":
                                pass2_j(pair, idx)
                            else:
                                ctx_j2(pair, idx)
                    if pair == MQ - 1:
                        for qc in range(S1):
                            outproj_qc(qc)

    nc.finalize()
    return nc


_NC_CACHE = {}


def get_nc(cfg: Cfg = Cfg()):
    if cfg not in _NC_CACHE:
        _NC_CACHE[cfg] = build_nc(cfg)
    return _NC_CACHE[cfg]


LAST_RESULT = None  # BassKernelResults of the most recent kernel() call


def make_in_maps(x, wq, bq, wk, bk, wv, bv, wo):
    scale = 1.0 / np.sqrt(np.float32(DEPTH))
    in_maps = []
    for core in range(N_CORES):
        b, g = divmod(core, GROUPS)
        sl = slice(g * DQ, (g + 1) * DQ)
        in_maps.append({
            "xt": np.ascontiguousarray(x[b].T),
            "wq": np.ascontiguousarray(wq[:, sl]) * scale,
            "bq": np.ascontiguousarray(bq[sl]) * scale,
            "wk": np.ascontiguousarray(wk[:, sl]),
            "bk": np.ascontiguousarray(bk[sl]),
            "wv": np.ascontiguousarray(wv[:, sl]),
            "bv": np.ascontiguousarray(bv[sl]),
            "wo": np.ascontiguousarray(wo[sl, :]),
        })
    return in_maps


def kernel(x, wq, bq, wk, bk, wv, bv, wo, bo, _trace=False):
    global LAST_RESULT
    arrs = [np.asarray(a, np.float32) for a in (x, wq, bq, wk, bk, wv, bv, wo)]
    bo = np.asarray(bo, np.float32)
    in_maps = make_in_maps(*arrs)

    nc = get_nc()
    res = run_bass_kernel_spmd(
        nc, in_maps, core_ids=list(range(N_CORES)), trace=_trace
    )
    LAST_RESULT = res

    attn = np.empty((B, H, S, S), np.float32)
    out = np.broadcast_to(bo, (B, S, D)).copy()
    for core in range(N_CORES):
        b, g = divmod(core, GROUPS)
        r = res.results[core]
        attn[b, g * HPC:(g + 1) * HPC] = r["attn_p"]
        out[b] += r["out_p"]
    return out, attn


# revision 34
# speedup vs baseline: 1.2994x; 1.0075x over previous
"""Multi-head spiking (ReLU) attention on 8 Trainium2 NeuronCores.

Sharding: core c -> (batch b = c//4, head-group g = c%4 of 4 heads).
Host pre-transposes x[b] and slices wq/wk/wv column-wise, wo row-wise.
Each core computes its 4 heads' attention matrix (one of the two graded
outputs) and a rank-256 partial of the output projection; the host sums
the four partials per batch and adds bo.

Device dataflow per core:
  phase 1 (fp32r matmuls): qT/kT [dq,S] projections (evicted to fp16),
    v [S,dq] (fp16).
  phase 2, interleaved per head-pair so attn-store DMA overlaps compute:
    pass1: logits[q,k] (row-paired K=64 fp16 matmuls, N=1024)
           -> ReLU evict -> attn output DMA   (DMA-heavy)
    pass2: logits[k,q] -> ReLU -> fp16 attnT  (PE/evict-heavy)
    ctx^T [dq,S]: v^T @ attnT, 2-head column tiling
  phase 3 (interleaved at the tail): out_part = ctx @ wo (fp16, N=1024).

PSUM is organised as [128,1024] two-bank units so each ACT/DVE eviction
covers 1024 elements (evictions and attn-store DMA are the two
bottleneck resources; TensorE alternates between them).
"""

from dataclasses import dataclass

import numpy as np

import concourse.bass as bass
import concourse.tile as tile
import concourse.mybir as mybir
from concourse import bacc
from concourse.bass_utils import run_bass_kernel_spmd

# Full-problem constants (hardcoded per harness contract).
B, S, D, H = 2, 2048, 1024, 16
DEPTH = D // H  # 64
N_CORES = 8
GROUPS = N_CORES // B       # 4 head-groups
HPC = H // GROUPS           # 4 heads per core
DQ = HPC * DEPTH            # 256 projected dims per core

F32 = mybir.dt.float32
F32R = mybir.dt.float32r
F16 = mybir.dt.float16
RELU = mybir.ActivationFunctionType.Relu


@dataclass(frozen=True)
class Cfg:
    s: int = S       # sequence length
    d: int = D       # model dim
    dq: int = DQ     # per-core projected dims (HPC*64)

    @property
    def ko(self):
        return self.d // 128   # contraction chunks for projections

    @property
    def mq(self):
        return self.dq // 128  # head-pair chunks (2 heads of depth 64 each)

    @property
    def s5(self):
        return self.s // 512

    @property
    def s1(self):
        return self.s // 128


class EvictBalancer:
    """Distribute PSUM->SBUF evictions between ScalarE and VectorE ~5:4
    (ratio of their element rates)."""

    def __init__(self, nc):
        self.nc = nc
        self.i = 0

    def _use_act(self):
        self.i += 1
        return self.i % 9 < 5

    def relu(self, out, psum, eng=None):
        use_act = self._use_act() if eng is None else (eng == "act")
        if use_act:
            self.nc.scalar.activation(out, psum, RELU)
        else:
            self.nc.vector.tensor_scalar_max(out, psum, 0.0)

    def copy(self, out, psum, eng=None):
        use_act = self._use_act() if eng is None else (eng == "act")
        if use_act:
            self.nc.scalar.copy(out, psum)
        else:
            self.nc.vector.tensor_copy(out, psum)


def build_nc(cfg: Cfg = Cfg()):
    s, d, dq = cfg.s, cfg.d, cfg.dq
    KO, MQ, S5, S1 = cfg.ko, cfg.mq, cfg.s5, cfg.s1
    S10 = s // 1024

    nc = bacc.Bacc(None, target_bir_lowering=False)

    # fp32r external inputs: bytes are plain fp32; the PE rounds on
    # consumption, and the dtype satisfies the fp32r-rounding verifier
    # without any casting DMA.
    xt = nc.dram_tensor("xt", [d, s], F32R, kind="ExternalInput")
    wq = nc.dram_tensor("wq", [d, dq], F32R, kind="ExternalInput")
    bq = nc.dram_tensor("bq", [dq], F32, kind="ExternalInput")
    wk = nc.dram_tensor("wk", [d, dq], F32R, kind="ExternalInput")
    bk = nc.dram_tensor("bk", [dq], F32, kind="ExternalInput")
    wv = nc.dram_tensor("wv", [d, dq], F32R, kind="ExternalInput")
    bv = nc.dram_tensor("bv", [dq], F32, kind="ExternalInput")
    wo = nc.dram_tensor("wo", [dq, d], F32, kind="ExternalInput")
    attn_p = nc.dram_tensor("attn_p", [2 * MQ, s, s], F32, kind="ExternalOutput")
    out_p = nc.dram_tensor("out_p", [s, d], F32, kind="ExternalOutput")

    with tile.TileContext(nc) as tc:
        ev = EvictBalancer(nc)
        with tc.tile_pool(name="persist", bufs=1) as pp:
            qT = pp.tile([128, MQ, s], F16)
            kT = pp.tile([128, MQ, s], F16)
            vv = pp.tile([128, S1, dq], F16)
            ctxT = pp.tile([128, MQ, s], F16)
            wo_sb = pp.tile([128, MQ, d], F16)
            bq_sb = pp.tile([128, MQ], F32)
            bk_sb = pp.tile([128, MQ], F32)
            bv_sb = pp.tile([128, dq], F32)


            # ---- Phase 1: projections (fp32r) ----
            with (
                tc.tile_pool(name="xw", bufs=1) as xw,
                tc.tile_pool(name="ps_proj", bufs=3, space="PSUM") as psp,
                tc.tile_pool(name="ps_vproj", bufs=2, space="PSUM") as psv,
            ):
                xt_sb = xw.tile([128, KO, s], F32R)
                wq_sb = xw.tile([128, KO, dq], F32R)
                wk_sb = xw.tile([128, KO, dq], F32R)
                wv_sb = xw.tile([128, KO, dq], F32R)
                # two token-half loads: projections for tokens 0..s/2
                # start while the second half streams in (one wait per
                # PSUM unit -- unlike a K-chunk split, which would stall
                # inside the accumulation loop)
                xt_r = xt.rearrange("(o p) t -> p o t", p=128)
                nc.sync.dma_start(out=xt_sb[:, :, :s // 2], in_=xt_r[:, :, :s // 2])
                nc.sync.dma_start(out=xt_sb[:, :, s // 2:], in_=xt_r[:, :, s // 2:])
                nc.gpsimd.dma_start(
                    out=wq_sb, in_=wq.rearrange("(o p) m -> p o m", p=128)
                )
                nc.gpsimd.dma_start(
                    out=wk_sb, in_=wk.rearrange("(o p) m -> p o m", p=128)
                )
                nc.gpsimd.dma_start(
                    out=wv_sb, in_=wv.rearrange("(o p) m -> p o m", p=128)
                )
                nc.gpsimd.dma_start(out=bq_sb, in_=bq.rearrange("(c p) -> p c", p=128))
                nc.gpsimd.dma_start(out=bk_sb, in_=bk.rearrange("(c p) -> p c", p=128))
                nc.gpsimd.dma_start(out=bv_sb, in_=bv[None, :].to_broadcast([128, dq]))
                # wo cast fp32->fp16 during the (SWDGE) load
                nc.gpsimd.dma_start(
                    out=wo_sb, in_=wo.rearrange("(c p) e -> p c e", p=128)
                )

                # qT / kT: [dq-part, tokens] fp16, 1024-wide PSUM units
                for w_sb, b_sb, dst in ((wq_sb, bq_sb, qT), (wk_sb, bk_sb, kT)):
                    for m in range(MQ):
                        for t in range(S10):
                            ps = psp.tile([128, 1024], F32, tag="proj")
                            for half in range(2):
                                tt = 2 * t + half
                                for o in range(KO):
                                    nc.tensor.matmul(
                                        ps[:, half * 512:(half + 1) * 512],
                                        lhsT=w_sb[:, o, m * 128:(m + 1) * 128],
                                        rhs=xt_sb[:, o, tt * 512:(tt + 1) * 512],
                                        start=(o == 0),
                                        stop=(o == KO - 1),
                                    )
                            nc.vector.tensor_scalar_add(
                                out=dst[:, m, t * 1024:(t + 1) * 1024],
                                in0=ps,
                                scalar1=b_sb[:, m, None],
                            )

                # v: [token-part, dq] fp16
                for t in range(S1):
                    ps = psv.tile([128, dq], F32, tag="vproj")
                    for o in range(KO):
                        nc.tensor.matmul(
                            ps,
                            lhsT=xt_sb[:, o, t * 128:(t + 1) * 128],
                            rhs=wv_sb[:, o, :],
                            start=(o == 0),
                            stop=(o == KO - 1),
                        )
                    nc.vector.tensor_tensor(
                        vv[:, t, :], ps, bv_sb, mybir.AluOpType.add
                    )

            # ---- Phase 2+3: attention, interleaved with output proj ----
            with (
                tc.tile_pool(name="ps_att", bufs=4, space="PSUM") as psa,
                tc.tile_pool(name="attn_rows", bufs=5) as rowpool,
                tc.tile_pool(name="attnT", bufs=2) as atpool,
                tc.tile_pool(name="out_rows", bufs=3) as outpool,
            ):
                at_tiles = {}

                def pass1_qc(pair, qc):
                    # one row of 128 q-tokens for both heads of the pair;
                    # fp16 N=1024 matmuls, row-paired across heads
                    rows_sb = [
                        rowpool.tile([128, s], F32, tag=f"row{h2}",
                                     name=f"row{pair}_{qc}_{h2}")
                        for h2 in range(2)
                    ]
                    for kc in range(S10):
                        pss = [
                            psa.tile([128, 1024], F32, tag="att",
                                     name=f"p1_{pair}_{qc}_{kc}_{h2}")
                            for h2 in range(2)
                        ]
                        for half in range(2):
                            kk = 2 * kc + half
                            for h2 in range(2):
                                rr = slice(64 * h2, 64 * (h2 + 1))
                                for mh in range(2):
                                    # 64x64 array tiling: tiles (64h2, 64mh)
                                    # all four run concurrently
                                    q0 = qc * 128 + 64 * mh
                                    nc.tensor.matmul(
                                        pss[h2][64 * mh:64 * (mh + 1),
                                                half * 512:(half + 1) * 512],
                                        lhsT=qT[rr, pair, q0:q0 + 64],
                                        rhs=kT[rr, pair, kk * 512:(kk + 1) * 512],
                                    )
                        for h2 in range(2):
                            ev.relu(
                                rows_sb[h2][:, kc * 1024:(kc + 1) * 1024],
                                pss[h2],
                            )
                    for h2 in range(2):
                        nc.sync.dma_start(
                            out=attn_p[2 * pair + h2, qc * 128:(qc + 1) * 128, :],
                            in_=rows_sb[h2],
                        )

                def pass2_j(pair, j):
                    # attnT for q-slice j*512, all 16 k-chunks, both heads
                    at = atpool.tile([128, 2, S1, 512], F16, tag="at",
                                     name=f"at_{pair}_{j}")
                    at_tiles[(pair, j)] = at
                    for kc in range(S1 // 2):
                        pss = [
                            psa.tile([128, 1024], F32, tag="att",
                                     name=f"p2_{pair}_{j}_{kc}_{h2}")
                            for h2 in range(2)
                        ]
                        for half in range(2):
                            kk = 2 * kc + half
                            for h2 in range(2):
                                rr = slice(64 * h2, 64 * (h2 + 1))
                                for mh in range(2):
                                    k0 = kk * 128 + 64 * mh
                                    nc.tensor.matmul(
                                        pss[h2][64 * mh:64 * (mh + 1),
                                                half * 512:(half + 1) * 512],
                                        lhsT=kT[rr, pair, k0:k0 + 64],
                                        rhs=qT[rr, pair, j * 512:(j + 1) * 512],
                                    )
                        for h2 in range(2):
                            ev.relu(
                                at[:, h2, 2 * kc:2 * kc + 2, :].rearrange(
                                    "p a b -> p (a b)"
                                ),
                                pss[h2],
                            )

                def ctx_j2(pair, j2):
                    # one [128,1024] unit = two q-chunks of 512
                    ps = psa.tile([128, 1024], F32, tag="att",
                                  name=f"ctx_{pair}_{j2}")
                    for jj in range(2):
                        at = at_tiles.pop((pair, 2 * j2 + jj))
                        sl = slice(jj * 512, (jj + 1) * 512)
                        for kc in range(S1):
                            for h2 in range(2):
                                nc.tensor.matmul(
                                    ps[64 * h2:64 * (h2 + 1), sl],
                                    lhsT=vv[:, kc, pair * 128 + 64 * h2:
                                            pair * 128 + 64 * (h2 + 1)],
                                    rhs=at[:, h2, kc, :],
                                    start=(kc == 0),
                                    stop=(kc == S1 - 1),
                                )
                    ev.copy(ctxT[:, pair, j2 * 1024:(j2 + 1) * 1024], ps)

                def outproj_qc(qc):
                    orow = outpool.tile([128, d], F32, tag="orow",
                                        name=f"orow_{qc}")
                    EW = min(1024, d)
                    for eu in range(d // EW):
                        ps = psa.tile([128, EW], F32, tag="att",
                                      name=f"po_{qc}_{eu}")
                        for e2 in range(EW // 512):
                            e = (EW // 512) * eu + e2
                            for m in range(MQ):
                                nc.tensor.matmul(
                                    ps[:, e2 * 512:(e2 + 1) * 512],
                                    lhsT=ctxT[:, m, qc * 128:(qc + 1) * 128],
                                    rhs=wo_sb[:, m, e * 512:(e + 1) * 512],
                                    start=(m == 0),
                                    stop=(m == MQ - 1),
                                )
                        ev.copy(orow[:, eu * EW:(eu + 1) * EW], ps)
                    nc.sync.dma_start(
                        out=out_p[qc * 128:(qc + 1) * 128, :], in_=orow
                    )

                # Interleave: pass1 paces the DMA; pass2/ctx/outproj slot
                # between its q-chunks to keep PE/ACT/DVE busy.
                for pair in range(MQ):
                    blocks = []
                    for j in range(S5):
                        blocks.append(("p2", j))
                        if j % 2 == 1:
                            blocks.append(("ctx", j // 2))
                    emitted = 0
                    for qc in range(S1):
                        pass1_qc(pair, qc)
                        want = (qc + 1) * len(blocks) // S1
                        while emitted < want:
                            kind, idx = blocks[emitted]
                            emitted += 1
                            if kind == "p2":
                                pass2_j(pair, idx)
                            else:
                                ctx_j2(pair, idx)
                    if pair == MQ - 1:
                        for qc in range(S1):
                            outproj_qc(qc)

    nc.finalize()
    return nc


_NC_CACHE = {}


def get_nc(cfg: Cfg = Cfg()):
    if cfg not in _NC_CACHE:
        _NC_CACHE[cfg] = build_nc(cfg)
    return _NC_CACHE[cfg]


LAST_RESULT = None  # BassKernelResults of the most recent kernel() call


def make_in_maps(x, wq, bq, wk, bk, wv, bv, wo):
    scale = 1.0 / np.sqrt(np.float32(DEPTH))
    in_maps = []
    for core in range(N_CORES):
        b, g = divmod(core, GROUPS)
        sl = slice(g * DQ, (g + 1) * DQ)
        in_maps.append({
            "xt": np.ascontiguousarray(x[b].T),
            "wq": np.ascontiguousarray(wq[:, sl]) * scale,
            "bq": np.ascontiguousarray(bq[sl]) * scale,
            "wk": np.ascontiguousarray(wk[:, sl]),
            "bk": np.ascontiguousarray(bk[sl]),
            "wv": np.ascontiguousarray(wv[:, sl]),
            "bv": np.ascontiguousarray(bv[sl]),
            "wo": np.ascontiguousarray(wo[sl, :]),
        })
    return in_maps


def kernel(x, wq, bq, wk, bk, wv, bv, wo, bo, _trace=False):
    global LAST_RESULT
    arrs = [np.asarray(a, np.float32) for a in (x, wq, bq, wk, bk, wv, bv, wo)]
    bo = np.asarray(bo, np.float32)
    in_maps = make_in_maps(*arrs)

    nc = get_nc()
    res = run_bass_kernel_spmd(
        nc, in_maps, core_ids=list(range(N_CORES)), trace=_trace
    )
    LAST_RESULT = res

    attn = np.empty((B, H, S, S), np.float32)
    out = np.broadcast_to(bo, (B, S, D)).copy()
    for core in range(N_CORES):
        b, g = divmod(core, GROUPS)
        r = res.results[core]
        attn[b, g * HPC:(g + 1) * HPC] = r["attn_p"]
        out[b] += r["out_p"]
    return out, attn
